# revision 1
# baseline (speedup 1.0000x reference)
"""Trainium2 Bass kernel for nn_EnhancedDepthwiseSeparableFFN.

Data-parallel over the batch: 8 samples -> 8 NeuronCores, one sample each.
Cross-core traffic: three tiny AllReduces for the BatchNorm batch statistics.

Per-core layout strategy (sample = (1024 hw, 256 d), expanded C=1024):
  - channel-major (C on partitions, HW on free) primary layout
  - dynamic 3x3 depthwise conv as a banded spatial matmul: three 128x128
    band tiles L_delta built on device from the per-sample softmax weights,
    stencil computed as yg_sp^T @ L blocks directly into channel-major PSUM;
    BN1's affine commutes through the stencil (per-channel scale rides the
    GELU pass via ACT's per-partition scale; the shift becomes a rank-1
    s (x) (b/a) term accumulated into PSUM as K=1 matmuls).
  - channel attention FCs / spatial 7x7 conv as small PE matmuls (banded
    matrices for the 7x7 prebuilt host-side from sw).
"""
import numpy as np

import concourse.bass as bass
import concourse.bacc as bacc
import concourse.tile as tile
from concourse import mybir, bass_utils, bass_isa

F32 = mybir.dt.float32
BF16 = mybir.dt.bfloat16
F32R = mybir.dt.float32r
AF = mybir.ActivationFunctionType
OP = mybir.AluOpType

D = 256          # model dim
C = 1024         # expanded channels
H = W = 32
HW = 1024
NCORES = 8
B = 8            # batch
EPS = 1e-5
CT = C // 128    # 8 channel tiles
HT = HW // 128   # 8 spatial tiles
NB = B * HW      # BN normalizer over (batch, h, w) = 8192 (8-core SPMD)


# ---------------------------------------------------------------- host consts

def _stencil_masks():
    """(128, 15*128) f32: columns = [d-1 q6..8 | d0 q0..q8 | d+1 q0..q2].

    L_delta[k_in, m_out] = kw[q],  q = (dh+1)*3 + (dw+1),
    dh = h_in - h_out = r_in - r_out - 4*delta, dw = w_in - w_out.
    """
    k = np.arange(128)
    m = np.arange(128)
    r_in, w_in = k // 32, k % 32
    r_out, w_out = m // 32, m % 32
    dw = w_in[:, None] - w_out[None, :]
    tiles = []
    specs = [(-1, range(6, 9)), (0, range(9)), (1, range(3))]
    for delta, qs in specs:
        dh = r_in[:, None] - r_out[None, :] - 4 * delta
        for q in qs:
            dh_q, dw_q = q // 3 - 1, q % 3 - 1
            tiles.append(((dh == dh_q) & (dw == dw_q)).astype(np.float32))
    return np.concatenate(tiles, axis=1)  # (128, 15*128)


def _spatial_bands(sw):
    """(128, 6*128) f32 lhsT band tiles for the 7x7 conv, cols =
    [ch0 d-1,d0,d+1 | ch1 d-1,d0,d+1].

    B_lhsT[k_in, m_out] = sw[0, ch, dh+3, dw+3], dh = h_in - h_out,
    dw = w_in - w_out, |dh|<=3, |dw|<=3, dh = r_in - r_out - 4*delta.
    """
    k = np.arange(128)
    m = np.arange(128)
    r_in, w_in = k // 32, k % 32
    r_out, w_out = m // 32, m % 32
    dw = w_in[:, None] - w_out[None, :]
    wok = np.abs(dw) <= 3
    tiles = []
    for ch in range(2):
        for delta in (-1, 0, 1):
            dh = r_in[:, None] - r_out[None, :] - 4 * delta
            hok = np.abs(dh) <= 3
            t = np.zeros((128, 128), np.float32)
            ok = hok & wok
            t[ok] = sw[0, ch][(dh[ok] + 3, dw[ok] + 3)]
            tiles.append(t)
    return np.concatenate(tiles, axis=1)  # (128, 6*128)


# ---------------------------------------------------------------- the program

def build_program(sw, sim_gelu_identity=False, n_cores=NCORES, debug=False):
    """Build + compile the SPMD program (identical on all 8 cores)."""
    gelu_f = AF.Identity if sim_gelu_identity else AF.Gelu

    nc = bacc.Bacc("TRN2", target_bir_lowering=False, debug=False,
                   num_devices=n_cores)

    # runtime inputs (per core)
    I = {}
    for name, shape in [
        ("x", [HW, D]),
        ("b1c", [128, CT]),
        ("g1c", [128, CT]), ("be1c", [128, CT]),
        ("g2c", [128, CT]), ("be2c", [128, CT]),
        ("aw1t", [C, 128]), ("ab1c", [128, 1]),
        ("aw2t", [128, 9]), ("ab2r", [1, 9]),
        ("caw1t", [C, 64]), ("caw2t", [64, C]),
        ("pwt", [C, D]),
        ("g3r", [1, D]), ("be3r", [1, D]), ("pbr", [1, D]),
        ("sbr", [1, 1]),
    ]:
        I[name] = nc.dram_tensor(name, shape, F32, kind="ExternalInput")
    for name, shape in [("xt", [D, HW]), ("w1t", [D, C])]:
        I[name] = nc.dram_tensor(name, shape, F32R, kind="ExternalInput")
    out_d = nc.dram_tensor("out", [HW, D], F32, kind="ExternalOutput")

    # compile-time consts
    ident = nc.inline_tensor(np.eye(128, dtype=np.float32), name="cident")
    ident1 = nc.inline_tensor(np.ones((1, 1), np.float32), name="cident1")
    ones_row = nc.inline_tensor(np.ones((1, 128), np.float32), name="conesr")
    ones_col = nc.inline_tensor(np.ones((128, 1), np.float32), name="conesc")
    invc_col = nc.inline_tensor(np.full((128, 1), 1.0 / C, np.float32),
                                name="cinvc")
    masks_c = nc.inline_tensor(_stencil_masks(), name="cmasks")
    spb_c = nc.inline_tensor(_spatial_bands(sw), name="cspb")

    dbg = {}
    if debug:
        for nm, shape in [("d_yg", [128, CT * HW]), ("d_stat1l", [128, 16]),
                          ("d_a1", [128, CT]), ("d_b1n", [128, CT]),
                          ("d_kw9", [1, 9]), ("d_L", [128, 384]),
                          ("d_srow", [1, HW]), ("d_ygsp", [128, HT * C]),
                          ("d_g2o", [128, CT * HW]), ("d_stat2l", [128, 16]),
                          ("d_scol", [128, CT]), ("d_proj", [128, 8 * D]),
                          ("d_avgpx", [128, HT]), ("d_mxpx", [128, HT]),
                          ("d_spcol", [128, HT]), ("d_spp", [128, 8 * D]),
                          ("d_stat3l", [1, 2 * D]), ("d_b1row", [1, C]),
                          ("d_mxc", [128, CT]), ("d_gapn", [128, CT])]:
            dbg[nm] = nc.dram_tensor(nm, shape, F32, kind="ExternalOutput")
    with tile.TileContext(nc) as tc:
        _body(nc, tc, I, out_d, ident, ident1, ones_row, ones_col, invc_col,
              masks_c, spb_c, gelu_f, n_cores, dbg)
    nc.compile()
    return nc


def _body(nc, tc, I, out_d, ident, ident1, ones_row, ones_col, invc_col,
          masks_c, spb_c, gelu_f, n_cores=NCORES, dbg=None):
    nb = n_cores * HW  # BN normalizer: one sample per participating core
    dbg = dbg or {}

    def tap(name, t):
        if name in dbg:
            nc.sync.dma_start(dbg[name].ap(), t)

    with tc.tile_pool(name="sb", bufs=1) as sb, \
         tc.tile_pool(name="sb2", bufs=1) as sb2, \
         tc.tile_pool(name="psb", bufs=3, space="PSUM") as psB, \
         tc.tile_pool(name="psh", bufs=2, space="PSUM") as psH, \
         tc.tile_pool(name="dram", bufs=6, space="DRAM") as dram:

        # ---------------- persistent SBUF tensors + loads
        def load(name, shape, ap_in, tag=None, pool=sb, view=None, q=None):
            t = pool.tile(shape, F32, tag=tag or name)
            dst = t[:] if view is None else t[:].rearrange(*view[0], **view[1])
            (q or nc.sync).dma_start(dst, ap_in)
            return t

        xt_sb = sb.tile([128, 2 * HW], F32R, tag="xt_sb")
        w1t_sb = sb.tile([128, 2 * C], F32R, tag="w1t_sb")
        for k in range(2):
            nc.sync.dma_start(
                xt_sb[:, k * HW:(k + 1) * HW],
                I["xt"].ap().rearrange("(k p) n -> p k n", p=128)[:, k, :])
            nc.sync.dma_start(
                w1t_sb[:, k * C:(k + 1) * C],
                I["w1t"].ap().rearrange("(k p) n -> p k n", p=128)[:, k, :])
        b1c = load("b1c", [128, CT], I["b1c"].ap())
        g1c = load("g1c", [128, CT], I["g1c"].ap())
        be1c = load("be1c", [128, CT], I["be1c"].ap())
        g2c = load("g2c", [128, CT], I["g2c"].ap())
        be2c = load("be2c", [128, CT], I["be2c"].ap())
        aw1t_sb = load("aw1t_sb", [128, 8 * 128],
                       I["aw1t"].ap().rearrange("(k p) n -> p k n", p=128),
                       view=(("p (k n) -> p k n",), dict(k=8)))
        ab1c = load("ab1c", [128, 1], I["ab1c"].ap())
        aw2t_sb = load("aw2t_sb", [128, 9], I["aw2t"].ap())
        ab2r = load("ab2r", [1, 9], I["ab2r"].ap())
        caw1t_sb = load("caw1t_sb", [128, 8 * 64],
                        I["caw1t"].ap().rearrange("(k p) n -> p k n", p=128),
                        view=(("p (k n) -> p k n",), dict(k=8)))
        caw2t_sb = load("caw2t_sb", [64, C], I["caw2t"].ap())
        paug = sb.tile([128, 8 * 260], F32, tag="paug")
        nc.sync.dma_start(
            paug[:].rearrange("p (k n) -> p k n", n=260)[:, :, 0:D],
            I["pwt"].ap().rearrange("(k p) n -> p k n", p=128))
        nc.vector.memset(
            paug[:].rearrange("p (k n) -> p k n", n=260)[:, :, D:D + 1],
            1.0 / C)
        nc.vector.memset(
            paug[:].rearrange("p (k n) -> p k n", n=260)[:, :, D + 1:D + 2],
            0.0)
        g3r = load("g3r", [1, D], I["g3r"].ap())
        be3r = load("be3r", [1, D], I["be3r"].ap())
        pbr = load("pbr", [1, D], I["pbr"].ap())
        sbr = load("sbr", [1, 1], I["sbr"].ap())
        tid = load("tid", [128, 128], ident.ap())
        tid1 = load("tid1", [1, 1], ident1.ap())
        tid1b = sb.tile([1, 1], F32, tag="tid1b")
        nc.vector.tensor_copy(tid1b[:], tid1[:])
        tonesr = load("tonesr", [1, 128], ones_row.ap())
        tonesrb = sb.tile([1, 128], F32R, tag="tonesrb")
        nc.vector.tensor_copy(tonesrb[:], tonesr[:])
        tonesc = load("tonesc", [128, 1], ones_col.ap())
        tinvc = load("tinvc", [128, 1], invc_col.ap())
        xres = load("xres", [128, 8 * D],
                    I["x"].ap().rearrange("(t p) d -> p t d", p=128),
                    view=(("p (t d) -> p t d",), dict(t=8)))
        masks = load("masks", [128, 15 * 128], masks_c.ap())
        spb = load("spb", [128, 6 * 128], spb_c.ap())

        # big working tensors
        yg = sb2.tile([128, CT * HW], F32, tag="yg")        # gelu1 out (ch-major)
        yg_sp = sb2.tile([128, HT * C], F32R, tag="ygsp")   # transposed, f32r
        g2o = sb2.tile([128, CT * HW], F32R, tag="yg")      # gelu2 out (f32r, reuses yg slot)
        yca = sb2.tile([128, CT * HW], F32R, tag="ygsp")    # ch-att out (reuses yg_sp slot)
        stat1l = sb.tile([128, 16], F32, tag="stat1l")      # packed local stats
        stat1g = sb.tile([128, 16], F32, tag="stat1g")      # after AllReduce
        stat2l = sb.tile([128, 16], F32, tag="stat2l")
        stat2g = sb.tile([128, 16], F32, tag="stat2g")
        scr_v = sb.tile([128, HW], F32, tag="scr_v")        # DVE STT scratch
        scr_g = sb.tile([128, 512], F32, tag="scr_g")       # GPSIMD STT scratch

        def halves(t, base):  # two (128,512) free slices of a 1024 chunk
            return [t[:, base + h * 512: base + (h + 1) * 512] for h in range(2)]

        # ============================ PHASE 1: expand + gelu1 + stats1
        for m in range(CT):
            ps = psB.tile([128, HW], F32, tag="psb")
            for h in range(2):
                for k in range(2):
                    nc.tensor.matmul(
                        ps[:, h * 512:(h + 1) * 512],
                        w1t_sb[:, k * C + m * 128: k * C + (m + 1) * 128],
                        xt_sb[:, k * HW + h * 512: k * HW + (h + 1) * 512],
                        start=(k == 0), stop=(k == 1))
            nc.scalar.activation(
                yg[:, m * HW:(m + 1) * HW], ps[:],
                gelu_f, bias=b1c[:, m:m + 1], scale=1.0,
                accum_out=stat1l[:, m:m + 1])
            srcg = yg[:, m * HW:(m + 1) * HW]
            nc.vector.scalar_tensor_tensor(
                scr_v[:], srcg, 0.0, srcg, OP.bypass, OP.mult,
                accum_out=stat1l[:, 8 + m:9 + m])

        tap("d_yg", yg[:])

        # ---- overlap AR1: forward transposes yg -> yg_sp
        for ht in range(HT):
            for grp in range(2):
                pt = psH.tile([128, 512], F32, tag="psh")
                for j in range(4):
                    cb = grp * 4 + j
                    nc.tensor.transpose(
                        pt[:, j * 128:(j + 1) * 128],
                        yg[:, cb * HW + ht * 128: cb * HW + (ht + 1) * 128],
                        tid[:])
                dst = yg_sp[:, ht * C + grp * 512: ht * C + (grp + 1) * 512]
                if grp == 0:
                    nc.vector.tensor_copy(dst, pt[:])
                else:
                    nc.scalar.copy(dst, pt[:])

        # ============================ AG1 (BN1 batch stats)
        # AllGather (floor ~4.6us vs AllReduce ~10-15us) + local combine
        bb1i = dram.tile([128, 16], F32, tag="bb1i")
        bb1o = dram.tile([n_cores * 128, 16], F32, tag="bb1o")
        nc.gpsimd.dma_start(bb1i[:], stat1l[:])
        nc.gpsimd.collective_compute(
            "AllGather", OP.bypass, replica_groups=[list(range(n_cores))],
            ins=[bb1i.opt()], outs=[bb1o.opt()])
        gath1 = sb.tile([128, n_cores * 16], F32, tag="gath1")
        nc.gpsimd.dma_start(
            gath1[:].rearrange("p (r f) -> p r f", f=16),
            bb1o[:].rearrange("(r p) f -> p r f", p=128))
        nc.vector.tensor_copy(stat1g[:], gath1[:, 0:16])
        for r in range(1, n_cores):
            nc.vector.tensor_tensor(stat1g[:], stat1g[:],
                                    gath1[:, r * 16:(r + 1) * 16], OP.add)

        # ============================ PHASE 3: BN1 affine + kw + L build
        def bn_affine(statg, gcol, becol, tagp):
            """-> (a, bn) per-channel scale/shift columns (128, CT)."""
            mns = sb.tile([128, CT], F32, tag=tagp + "m")
            ex2 = sb.tile([128, CT], F32, tag=tagp + "e")
            var = sb.tile([128, CT], F32, tag=tagp + "v")
            a = sb.tile([128, CT], F32, tag=tagp + "a")
            bn = sb.tile([128, CT], F32, tag=tagp + "b")
            nc.vector.tensor_scalar_mul(mns[:], statg[:, 0:8], 1.0 / nb)
            nc.vector.tensor_scalar_mul(ex2[:], statg[:, 8:16], 1.0 / nb)
            nc.vector.tensor_tensor(var[:], mns[:], mns[:], OP.mult)
            nc.vector.tensor_tensor(var[:], ex2[:], var[:], OP.subtract)
            nc.vector.tensor_scalar_add(var[:], var[:], EPS)
            nc.scalar.sqrt(var[:], var[:])
            nc.vector.reciprocal(a[:], var[:])
            nc.vector.tensor_tensor(a[:], gcol[:], a[:], OP.mult)
            nc.vector.tensor_tensor(bn[:], mns[:], a[:], OP.mult)
            nc.vector.tensor_tensor(bn[:], becol[:], bn[:], OP.subtract)
            return a, bn

        a1, b1n = bn_affine(stat1g, g1c, be1c, "s1")
        inva1 = sb.tile([128, CT], F32, tag="inva1")
        bpre = sb.tile([128, CT], F32, tag="bpre")
        nc.vector.reciprocal(inva1[:], a1[:])
        nc.vector.tensor_tensor(bpre[:], b1n[:], inva1[:], OP.mult)
        # b' as a (1, 1024) row on partition 0: PE transpose to (CT, 128),
        # then bounce through DRAM (engines cannot move data across partitions)
        psb1 = psH.tile([CT, 128], F32, tag="psh")
        nc.tensor.transpose(psb1[:], bpre[:], tid[:])
        b1pr8 = sb.tile([CT, 128], F32, tag="b1pr8")
        nc.scalar.copy(b1pr8[:], psb1[:])
        bpd = dram.tile([CT, 128], F32, tag="bpd")
        nc.gpsimd.dma_start(bpd[:], b1pr8[:])
        b1row = sb.tile([1, C], F32, tag="b1row")
        nc.gpsimd.dma_start(b1row[:], bpd[:].rearrange("c p -> (c p)")[None, :])
        b1rowb = sb.tile([1, C], F32R, tag="b1rowb")
        nc.vector.tensor_copy(b1rowb[:], b1row[:])

        tap("d_a1", a1[:])
        tap("d_b1n", b1n[:])
        tap("d_b1row", b1row[:])
        # gap (local, normalized) -> kw
        gapn = sb.tile([128, CT], F32, tag="gapn")
        nc.vector.tensor_scalar_mul(gapn[:], stat1l[:, 0:8], 1.0 / HW)
        nc.vector.tensor_tensor(gapn[:], gapn[:], a1[:], OP.mult)
        nc.vector.tensor_tensor(gapn[:], gapn[:], b1n[:], OP.add)

        ph1 = psH.tile([128, 1], F32, tag="psh")
        for k in range(CT):
            nc.tensor.matmul(ph1[:], aw1t_sb[:, k * 128:(k + 1) * 128],
                             gapn[:, k:k + 1], start=(k == 0), stop=(k == 7))
        h1 = sb.tile([128, 1], F32, tag="h1")
        nc.scalar.activation(h1[:], ph1[:], AF.Relu, bias=ab1c[:], scale=1.0)
        ps9 = psH.tile([1, 9], F32, tag="psh")
        nc.tensor.matmul(ps9[:], h1[:], aw2t_sb[:], start=True, stop=True)
        v9 = sb.tile([1, 9], F32, tag="v9")
        nc.vector.tensor_tensor(v9[:], ps9[:], ab2r[:], OP.add)
        mx9 = sb.tile([1, 1], F32, tag="mx9")
        nc.vector.tensor_reduce(mx9[:], v9[:], mybir.AxisListType.X, OP.max)
        nc.vector.tensor_scalar(v9[:], v9[:], mx9[:], None, OP.subtract)
        e9 = sb.tile([1, 9], F32, tag="e9")
        se = sb.tile([1, 1], F32, tag="se")
        nc.scalar.activation(e9[:], v9[:], AF.Exp, bias=0.0, scale=1.0,
                             accum_out=se[:])
        rse = sb.tile([1, 1], F32, tag="rse")
        nc.vector.reciprocal(rse[:], se[:])
        kw9 = sb.tile([1, 9], F32, tag="kw9")
        nc.vector.tensor_scalar(kw9[:], e9[:], rse[:], None, OP.mult)
        tap("d_kw9", kw9[:])
        tap("d_gapn", gapn[:])
        # broadcast kw to all partitions
        pskb = psH.tile([128, 9], F32, tag="psh")
        nc.tensor.matmul(pskb[:], tonesr[:], kw9[:], start=True, stop=True)
        kwb = sb.tile([128, 9], F32, tag="kwb")
        nc.scalar.copy(kwb[:], pskb[:])

        # L band tiles, concatenated [L(-1) | L(0) | L(+1)] to allow wide-N
        # stencil matmuls with shifted output windows
        L = sb.tile([128, 3 * 128], F32, tag="L")
        Ldm, Ld0, Ldp = L[:, 0:128], L[:, 128:256], L[:, 256:384]
        nc.vector.tensor_scalar(Ldm, masks[:, 0:128], kwb[:, 6:7], None, OP.mult)
        for i, q in enumerate((7, 8)):
            nc.vector.scalar_tensor_tensor(
                Ldm, masks[:, (1 + i) * 128:(2 + i) * 128], kwb[:, q:q + 1],
                Ldm, OP.mult, OP.add)
        nc.vector.tensor_scalar(Ld0, masks[:, 3 * 128:4 * 128], kwb[:, 0:1],
                                None, OP.mult)
        for q in range(1, 9):
            nc.vector.scalar_tensor_tensor(
                Ld0, masks[:, (3 + q) * 128:(4 + q) * 128], kwb[:, q:q + 1],
                Ld0, OP.mult, OP.add)
        nc.vector.tensor_scalar(Ldp, masks[:, 12 * 128:13 * 128], kwb[:, 0:1],
                                None, OP.mult)
        for i, q in enumerate((1, 2)):
            nc.vector.scalar_tensor_tensor(
                Ldp, masks[:, (13 + i) * 128:(14 + i) * 128], kwb[:, q:q + 1],
                Ldp, OP.mult, OP.add)

        tap("d_L", L[:])
        Lb = sb.tile([128, 3 * 128], F32R, tag="Lb")
        nc.vector.tensor_copy(Lb[:], L[:])

        # s row: per-pixel sum of present taps (column sums of L blocks)
        psu = psH.tile([1, 384], F32, tag="psh")
        for i in range(3):
            nc.tensor.matmul(psu[:, i * 128:(i + 1) * 128], tonesc[:],
                             L[:, i * 128:(i + 1) * 128], start=True, stop=True)
        u_sb = sb.tile([1, 384], F32, tag="u_sb")
        nc.scalar.copy(u_sb[:], psu[:])
        um, u0, up = u_sb[:, 0:128], u_sb[:, 128:256], u_sb[:, 256:384]
        srow = sb.tile([1, HW], F32, tag="srow")
        si = sb.tile([1, 128], F32, tag="si")
        nc.vector.tensor_tensor(si[:], u0, up, OP.add)
        nc.vector.tensor_tensor(si[:], si[:], um, OP.add)
        nc.vector.tensor_tensor(srow[:, 0:128], u0, um, OP.add)       # t_out=0
        nc.vector.tensor_tensor(srow[:, 896:1024], u0, up, OP.add)    # t_out=7
        for t in range(1, 7):
            nc.vector.tensor_copy(srow[:, t * 128:(t + 1) * 128], si[:])
        srowb = sb.tile([1, HW], F32R, tag="srowb")
        nc.vector.tensor_copy(srowb[:], srow[:])

        tap("d_srow", srow[:])
        tap("d_ygsp", yg_sp[:])
        # ============================ PHASE 4: stencil + gelu2 + stats2
        # stencil: per channel-tile, one (128,1024) psum; rank-1 opens each
        # bank's accumulation group, then one wide-N matmul per t_in covering
        # its t_out window [(t_in-1)*128, (t_in+2)*128) -- split at the bank
        # boundary (PSUM matmuls may not cross banks).
        for c in range(CT):
            psz = psB.tile([128, HW], F32, tag="psb")
            for h in range(2):
                nc.tensor.matmul(psz[:, h * 512:(h + 1) * 512],
                                 b1rowb[:, c * 128:(c + 1) * 128],
                                 srowb[:, h * 512:(h + 1) * 512],
                                 start=True, stop=False)
            for t_in in range(HT):
                lo = max(0, (t_in - 1) * 128)
                hi = min(HW, (t_in + 2) * 128)
                # rhs window of Lcat aligned so column block t_out uses
                # L_{t_out - t_in}
                roff = 128 + (lo - t_in * 128)   # Lcat offset for out col lo
                if lo < 512 < hi:
                    pieces = [(lo, 512), (512, hi)]
                else:
                    pieces = [(lo, hi)]
                for (a, b) in pieces:
                    ra = roff + (a - lo)
                    last_bank0 = (a < 512) and (t_in == 4)
                    last_bank1 = (a >= 512) and (t_in == 7)
                    nc.tensor.matmul(
                        psz[:, a:b],
                        yg_sp[:, t_in * C + c * 128: t_in * C + (c + 1) * 128],
                        Lb[:, ra:ra + (b - a)],
                        start=False, stop=(last_bank0 or last_bank1))
            nc.scalar.activation(
                g2o[:, c * HW:(c + 1) * HW], psz[:], gelu_f,
                bias=0.0, scale=a1[:, c:c + 1],
                accum_out=stat2l[:, c:c + 1])
            srcg2 = g2o[:, c * HW:(c + 1) * HW]
            nc.vector.scalar_tensor_tensor(
                scr_v[:], srcg2, 0.0, srcg2, OP.bypass, OP.mult,
                accum_out=stat2l[:, 8 + c:9 + c])

        tap("d_g2o", g2o[:])
        tap("d_stat2l", stat2l[:])
        # ============================ AG2 (BN2 batch stats)
        bb2i = dram.tile([128, 16], F32, tag="bb2i")
        bb2o = dram.tile([n_cores * 128, 16], F32, tag="bb2o")
        nc.gpsimd.dma_start(bb2i[:], stat2l[:])
        nc.gpsimd.collective_compute(
            "AllGather", OP.bypass, replica_groups=[list(range(n_cores))],
            ins=[bb2i.opt()], outs=[bb2o.opt()])
        gath2 = sb.tile([128, n_cores * 16], F32, tag="gath2")
        nc.gpsimd.dma_start(
            gath2[:].rearrange("p (r f) -> p r f", f=16),
            bb2o[:].rearrange("(r p) f -> p r f", p=128))
        nc.vector.tensor_copy(stat2g[:], gath2[:, 0:16])
        for r in range(1, n_cores):
            nc.vector.tensor_tensor(stat2g[:], stat2g[:],
                                    gath2[:, r * 16:(r + 1) * 16], OP.add)

        # overlap AR2: per-channel max over HW of g2o (local)
        mxc = sb.tile([128, CT], F32, tag="mxc")
        for c in range(CT):
            nc.vector.tensor_reduce(mxc[:, c:c + 1],
                                    g2o[:, c * HW:(c + 1) * HW],
                                    mybir.AxisListType.X, OP.max)

        # ============================ PHASE 6: BN2 + channel attention
        a2, b2n = bn_affine(stat2g, g2c, be2c, "s2")
        amx = sb.tile([128, 2 * CT], F32, tag="amx")
        nc.vector.tensor_scalar_mul(amx[:, 0:8], stat2l[:, 0:8], 1.0 / HW)
        nc.vector.tensor_tensor(amx[:, 0:8], amx[:, 0:8], a2[:], OP.mult)
        nc.vector.tensor_tensor(amx[:, 0:8], amx[:, 0:8], b2n[:], OP.add)
        nc.vector.tensor_tensor(amx[:, 8:16], mxc[:], a2[:], OP.mult)
        nc.vector.tensor_tensor(amx[:, 8:16], amx[:, 8:16], b2n[:], OP.add)

        psf = psH.tile([64, 2], F32, tag="psh")
        for k in range(CT):
            nc.tensor.matmul(psf[:], caw1t_sb[:, k * 64:(k + 1) * 64],
                             amx[:, k:k + 9:8], start=(k == 0), stop=(k == 7))
        hp = sb.tile([64, 2], F32, tag="hp")
        nc.scalar.activation(hp[:], psf[:], AF.Relu, bias=0.0, scale=1.0)
        hsum = sb.tile([64, 1], F32, tag="hsum")
        nc.vector.tensor_tensor(hsum[:], hp[:, 0:1], hp[:, 1:2], OP.add)

        scol = sb.tile([128, CT], F32, tag="scol")
        for c in range(CT):
            pss = psH.tile([128, 1], F32, tag="psh")
            nc.tensor.matmul(pss[:], caw2t_sb[:, c * 128:(c + 1) * 128],
                             hsum[:], start=True, stop=True)
            nc.scalar.activation(scol[:, c:c + 1], pss[:], AF.Sigmoid,
                                 bias=0.0, scale=1.0)
        tap("d_scol", scol[:])
        tap("d_mxc", mxc[:])
        sprime = sb.tile([128, CT], F32, tag="sprime")
        b2s = sb.tile([128, CT], F32, tag="b2s")
        nc.vector.tensor_tensor(sprime[:], scol[:], a2[:], OP.mult)
        nc.vector.tensor_tensor(b2s[:], scol[:], b2n[:], OP.mult)

        # y_ca (materialized for the channel-max + channel-mean)
        for c in range(CT):
            nc.vector.tensor_scalar(yca[:, c * HW:(c + 1) * HW],
                                    g2o[:, c * HW:(c + 1) * HW],
                                    sprime[:, c:c + 1], b2s[:, c:c + 1],
                                    OP.mult, OP.add)

        # scaled projection weights
        pws = sb2.tile([128, 8 * 260], F32R, tag="pws")
        for c in range(CT):
            nc.vector.tensor_scalar(pws[:, c * 260:c * 260 + 258],
                                    paug[:, c * 260:c * 260 + 258],
                                    sprime[:, c:c + 1], None, OP.mult)
        # t2 row (rank-1 bias of the projection)
        pst2 = psH.tile([1, 258], F32, tag="psh")
        for c in range(CT):
            nc.tensor.matmul(pst2[:], b2s[:, c:c + 1],
                             paug[:, c * 260:c * 260 + 258],
                             start=(c == 0), stop=(c == 7))
        u2row = sb.tile([1, 258], F32R, tag="u2row")
        nc.scalar.copy(u2row[:], pst2[:])

        # projection -> proj_sb (spatial-major (hw, d))
        proj_sb = sb2.tile([128, 8 * D], F32, tag="proj_sb")
        avgpx = sb.tile([128, HT], F32, tag="avgpx")
        for mt in range(HT):
            psp = psH.tile([128, 258], F32, tag="psh")
            for c in range(CT):
                nc.tensor.matmul(psp[:],
                                 g2o[:, c * HW + mt * 128: c * HW + (mt + 1) * 128],
                                 pws[:, c * 260:c * 260 + 258],
                                 start=(c == 0), stop=False)
            nc.tensor.matmul(psp[:], tonesrb[:], u2row[:], start=False,
                             stop=True)
            dst = proj_sb[:, mt * D:(mt + 1) * D]
            if mt % 2 == 0:
                nc.vector.tensor_copy(dst, psp[:, 0:D])
            else:
                nc.scalar.copy(dst, psp[:, 0:D])
            nc.vector.tensor_copy(avgpx[:, mt:mt + 1], psp[:, D:D + 1])

        tap("d_proj", proj_sb[:])
        # channel max (per pixel): in-place pairwise tree over yca (destroys
        # it -- emitted after the channel-mean matmuls, WAR deps serialize),
        # then a partition all-reduce
        for i in range(4):
            nc.vector.tensor_tensor(yca[:, (2 * i) * HW:(2 * i + 1) * HW],
                                    yca[:, (2 * i) * HW:(2 * i + 1) * HW],
                                    yca[:, (2 * i + 1) * HW:(2 * i + 2) * HW],
                                    OP.max)
        nc.vector.tensor_tensor(yca[:, 0:HW], yca[:, 0:HW],
                                yca[:, 2 * HW:3 * HW], OP.max)
        nc.vector.tensor_tensor(yca[:, 4 * HW:5 * HW], yca[:, 4 * HW:5 * HW],
                                yca[:, 6 * HW:7 * HW], OP.max)
        nc.vector.tensor_tensor(yca[:, 0:HW], yca[:, 0:HW],
                                yca[:, 4 * HW:5 * HW], OP.max)
        mxbc = sb2.tile([128, HW], F32, tag="mxbc")
        nc.gpsimd.partition_all_reduce(mxbc[:], yca[:, 0:HW], 128,
                                       bass_isa.ReduceOp.max)
        # row 0 of mxbc = per-pixel channel max; to columns via PE transposes
        mxpx = sb.tile([128, HT], F32, tag="mxpx")
        psmx = psB.tile([128, HT], F32, tag="psb")
        for t in range(HT):
            nc.tensor.transpose(psmx[:, t:t + 1],
                                mxbc[0:1, t * 128:(t + 1) * 128], tid1b[:])
        nc.vector.tensor_copy(mxpx[:], psmx[:])

        tap("d_avgpx", avgpx[:])
        tap("d_mxpx", mxpx[:])
        # sb broadcast column
        pssb = psH.tile([128, 1], F32, tag="psh")
        nc.tensor.matmul(pssb[:], tonesr[:], sbr[:], start=True, stop=True)
        sbc = sb.tile([128, 1], F32, tag="sbc")
        nc.scalar.copy(sbc[:], pssb[:])

        # spatial 7x7 conv as 6 shifted-column matmuls (2 ch x 3 bands),
        # then one sigmoid over all 8 output columns
        pssp = psH.tile([128, HT], F32, tag="psh")
        mmspecs = []
        for ch, srccol in ((0, avgpx), (1, mxpx)):
            mmspecs.append((ch * 3 + 1, slice(0, 8), srccol[:, 0:8]))
            mmspecs.append((ch * 3 + 2, slice(1, 8), srccol[:, 0:7]))
            mmspecs.append((ch * 3 + 0, slice(0, 7), srccol[:, 1:8]))
        for i, (bi, osl, rhs) in enumerate(mmspecs):
            nc.tensor.matmul(pssp[:, osl], spb[:, bi * 128:(bi + 1) * 128],
                             rhs, start=(i == 0), stop=(i == len(mmspecs) - 1))
        spcol = sb.tile([128, HT], F32, tag="spcol")
        nc.scalar.activation(spcol[:], pssp[:], AF.Sigmoid, bias=sbc[:],
                             scale=1.0)

        # spp = proj * sp (spatial scale, per-partition)
        spp = sb2.tile([128, 8 * D], F32, tag="spp")
        for mt in range(HT):
            nc.vector.tensor_scalar(spp[:, mt * D:(mt + 1) * D],
                                    proj_sb[:, mt * D:(mt + 1) * D],
                                    spcol[:, mt:mt + 1], None, OP.mult)

        # BN3 stats: sum(sp*proj) and sum((sp*proj)^2) over hw
        pst3a = psH.tile([1, D], F32, tag="psh")
        for mt in range(HT):
            nc.tensor.matmul(pst3a[:], spcol[:, mt:mt + 1],
                             proj_sb[:, mt * D:(mt + 1) * D],
                             start=(mt == 0), stop=(mt == 7))
        pst3b = psH.tile([1, D], F32, tag="psh")
        sqs = sb.tile([128, 2 * D], F32, tag="sqs")
        for mt in range(HT):
            half = (mt % 2) * D
            nc.scalar.square(sqs[:, half:half + D], spp[:, mt * D:(mt + 1) * D])
            nc.tensor.matmul(pst3b[:], tonesc[:], sqs[:, half:half + D],
                             start=(mt == 0), stop=(mt == 7))
        stat3l = sb.tile([1, 2 * D], F32, tag="stat3l")
        nc.scalar.copy(stat3l[:, 0:D], pst3a[:])
        nc.vector.tensor_copy(stat3l[:, D:2 * D], pst3b[:])

        tap("d_spp", spp[:])
        tap("d_stat3l", stat3l[:])
        # ============================ AG3 (BN3 batch stats)
        bb3i = dram.tile([1, 2 * D], F32, tag="bb3i")
        bb3o = dram.tile([n_cores, 2 * D], F32, tag="bb3o")
        nc.gpsimd.dma_start(bb3i[:], stat3l[:])
        nc.gpsimd.collective_compute(
            "AllGather", OP.bypass, replica_groups=[list(range(n_cores))],
            ins=[bb3i.opt()], outs=[bb3o.opt()])
        gath3 = sb.tile([n_cores, 2 * D], F32, tag="gath3")
        nc.gpsimd.dma_start(gath3[:], bb3o[:])
        pst3g = psH.tile([1, 2 * D], F32, tag="psh")
        nc.tensor.matmul(pst3g[:], tonesc[0:n_cores, :], gath3[:],
                         start=True, stop=True)
        stat3g = sb.tile([1, 2 * D], F32, tag="stat3g")
        nc.scalar.copy(stat3g[:], pst3g[:])

        # BN3 affine in row form
        m3 = sb.tile([1, D], F32, tag="m3")
        v3 = sb.tile([1, D], F32, tag="v3")
        a3r = sb.tile([1, D], F32, tag="a3r")
        c3r = sb.tile([1, D], F32, tag="c3r")
        nc.vector.tensor_scalar_mul(m3[:], stat3g[:, 0:D], 1.0 / nb)
        nc.vector.tensor_scalar_mul(v3[:], stat3g[:, D:2 * D], 1.0 / nb)
        tmp3 = sb.tile([1, D], F32, tag="tmp3")
        nc.vector.tensor_tensor(tmp3[:], m3[:], m3[:], OP.mult)
        nc.vector.tensor_tensor(v3[:], v3[:], tmp3[:], OP.subtract)
        nc.vector.tensor_scalar_add(v3[:], v3[:], EPS)
        nc.scalar.sqrt(v3[:], v3[:])
        nc.vector.reciprocal(v3[:], v3[:])
        nc.vector.tensor_tensor(a3r[:], g3r[:], v3[:], OP.mult)
        # BN3 input is spp + pb, but pb cancels: c3 = be3 - a3 * mean(spp)
        nc.vector.tensor_tensor(tmp3[:], a3r[:], m3[:], OP.mult)
        nc.vector.tensor_tensor(c3r[:], be3r[:], tmp3[:], OP.subtract)

        # broadcast a3/c3 to all partitions
        a3b = sb.tile([128, D], F32, tag="a3b")
        c3b = sb.tile([128, D], F32, tag="c3b")
        for rowt, dstt in ((a3r, a3b), (c3r, c3b)):
            psx = psH.tile([128, D], F32, tag="psh")
            nc.tensor.matmul(psx[:], tonesr[:], rowt[:], start=True, stop=True)
            nc.vector.tensor_copy(dstt[:], psx[:])

        # final: out = (x + c3) + spp*a3, two DVE passes per tile
        out_sb = sb2.tile([128, 8 * D], F32, tag="outsb")
        for mt in range(HT):
            sl = slice(mt * D, (mt + 1) * D)
            nc.vector.tensor_tensor(out_sb[:, sl], xres[:, sl], c3b[:], OP.add)
        for mt in range(HT):
            sl = slice(mt * D, (mt + 1) * D)
            nc.vector.tensor_tensor(spp[:, sl], spp[:, sl], a3b[:], OP.mult)
            nc.vector.tensor_tensor(out_sb[:, sl], out_sb[:, sl], spp[:, sl],
                                    OP.add)
            nc.sync.dma_start(
                out_d.ap().rearrange("(t p) d -> p t d", p=128)[:, mt, :],
                out_sb[:, mt * D:(mt + 1) * D])


# ---------------------------------------------------------------- host driver

def shard_inputs(inputs):
    """Full inputs -> per-core in_maps (host-side layout staging only)."""
    x = np.ascontiguousarray(np.asarray(inputs["x"], np.float32))
    w1 = np.asarray(inputs["w1"], np.float32)
    shared = {
        "w1t": np.ascontiguousarray(w1.T),
        "b1c": np.ascontiguousarray(inputs["b1"].reshape(CT, 128).T),
        "g1c": np.ascontiguousarray(inputs["g1"].reshape(CT, 128).T),
        "be1c": np.ascontiguousarray(inputs["be1"].reshape(CT, 128).T),
        "g2c": np.ascontiguousarray(inputs["g2"].reshape(CT, 128).T),
        "be2c": np.ascontiguousarray(inputs["be2"].reshape(CT, 128).T),
        "aw1t": np.ascontiguousarray(np.asarray(inputs["aw1"], np.float32).T),
        "ab1c": np.ascontiguousarray(inputs["ab1"].reshape(1, 128).T),
        "aw2t": np.ascontiguousarray(np.asarray(inputs["aw2"], np.float32).T),
        "ab2r": np.ascontiguousarray(inputs["ab2"].reshape(1, 9)),
        "caw1t": np.ascontiguousarray(np.asarray(inputs["ca_w1"], np.float32).T),
        "caw2t": np.ascontiguousarray(np.asarray(inputs["ca_w2"], np.float32).T),
        "pwt": np.ascontiguousarray(np.asarray(inputs["pw"], np.float32).T),
        "g3r": np.ascontiguousarray(inputs["g3"].reshape(1, D)),
        "be3r": np.ascontiguousarray(inputs["be3"].reshape(1, D)),
        "pbr": np.ascontiguousarray(inputs["pb"].reshape(1, D)),
        "sbr": np.ascontiguousarray(inputs["sb"].reshape(1, 1)),
    }
    shared = {k: v.astype(np.float32) for k, v in shared.items()}
    in_maps = []
    for i in range(NCORES):
        m = dict(shared)
        m["x"] = np.ascontiguousarray(x[i])
        m["xt"] = np.ascontiguousarray(x[i].T)
        in_maps.append(m)
    return in_maps


_CACHE = {}


def get_program(sw, sim_gelu_identity=False, n_cores=NCORES, debug=False):
    key = ("sim" if sim_gelu_identity else "hw", n_cores, debug, sw.tobytes())
    if key not in _CACHE:
        _CACHE[key] = build_program(sw, sim_gelu_identity=sim_gelu_identity,
                                    n_cores=n_cores, debug=debug)
    return _CACHE[key]


def run(inputs, trace=False):
    nc = get_program(np.asarray(inputs["sw"], np.float32))
    in_maps = shard_inputs(inputs)
    r = bass_utils.run_bass_kernel_spmd(
        nc, in_maps, core_ids=list(range(NCORES)), trace=trace)
    out = np.stack([r.results[i]["out"] for i in range(NCORES)], axis=0)
    return out.astype(np.float32), r


def kernel(**inputs) -> np.ndarray:
    out, _ = run(inputs, trace=False)
    return out



# revision 9
# speedup vs baseline: 1.2072x; 1.2072x over previous
"""Trainium2 Bass kernel for nn_EnhancedDepthwiseSeparableFFN (v2).

Data-parallel over the batch: 8 samples -> 8 NeuronCores, one sample each.
Cross-core traffic: three tiny AllGathers for the BatchNorm batch statistics.

v2 changes over the baseline (243us):
  - bf16 everywhere on the PE path (FWL weight loads, full-rate stencil
    pieces, cheap LDWEIGHTS) and for the big elementwise tensors (DVE 2x/4x).
  - double-expand: the spatial-major gelu1 output is recomputed as a second
    expand matmul pass (overlapping the AG1 collective) instead of 64 PE
    transposes + PSUM copies.
  - stats ride fused ops: ACT accum (sums) + DVE tensor_tensor_reduce
    (sum-of-squares); per-channel max moved to the idle GPSIMD engine.
  - BN1/BN2 rsqrt via the bit-trick + 2 Newton steps on DVE (no ACT Sqrt
    table loads between the Gelu phases); ACT tables are preloaded with
    dummy ops so Exp/Gelu/Sigmoid loads hide under collective waits.
  - srow from a host-side tap-indicator matrix (one small matmul);
    b1row via 8 tiny column transposes (no DRAM bounce).
  - final tail split across DVE and GPSIMD.
"""
import numpy as np

import concourse.bass as bass
import concourse.bacc as bacc
import concourse.tile as tile
from concourse import mybir, bass_utils, bass_isa

F32 = mybir.dt.float32
BF16 = mybir.dt.bfloat16
U32 = mybir.dt.uint32
AF = mybir.ActivationFunctionType
OP = mybir.AluOpType

NP_BF16 = mybir.dt.np(BF16)

D = 256          # model dim
C = 1024         # expanded channels
H = W = 32
HW = 1024
NCORES = 8
B = 8            # batch
EPS = 1e-5
CT = C // 128    # 8 channel tiles
HT = HW // 128   # 8 spatial tiles
RSQRT_ITERS = 2


# ---------------------------------------------------------------- host consts

def _stencil_masks():
    """(128, 15*128) f32: columns = [d-1 q6..8 | d0 q0..q8 | d+1 q0..q2].

    L_delta[k_in, m_out] = kw[q],  q = (dh+1)*3 + (dw+1),
    dh = h_in - h_out = r_in - r_out - 4*delta, dw = w_in - w_out.
    """
    k = np.arange(128)
    m = np.arange(128)
    r_in, w_in = k // 32, k % 32
    r_out, w_out = m // 32, m % 32
    dw = w_in[:, None] - w_out[None, :]
    tiles = []
    specs = [(-1, range(6, 9)), (0, range(9)), (1, range(3))]
    for delta, qs in specs:
        dh = r_in[:, None] - r_out[None, :] - 4 * delta
        for q in qs:
            dh_q, dw_q = q // 3 - 1, q % 3 - 1
            tiles.append(((dh == dh_q) & (dw == dw_q)).astype(np.float32))
    return np.concatenate(tiles, axis=1)  # (128, 15*128)


def _spatial_bands(sw):
    """(128, 6*128) f32 lhsT band tiles for the 7x7 conv, cols =
    [ch0 d-1,d0,d+1 | ch1 d-1,d0,d+1]."""
    k = np.arange(128)
    m = np.arange(128)
    r_in, w_in = k // 32, k % 32
    r_out, w_out = m // 32, m % 32
    dw = w_in[:, None] - w_out[None, :]
    wok = np.abs(dw) <= 3
    tiles = []
    for ch in range(2):
        for delta in (-1, 0, 1):
            dh = r_in[:, None] - r_out[None, :] - 4 * delta
            hok = np.abs(dh) <= 3
            t = np.zeros((128, 128), np.float32)
            ok = hok & wok
            t[ok] = sw[0, ch][(dh[ok] + 3, dw[ok] + 3)]
            tiles.append(t)
    return np.concatenate(tiles, axis=1)  # (128, 6*128)


def _tap_counts():
    """(9, 1024) f32: SB9[q, px] = 1 if 3x3 tap q is in-bounds at pixel px.

    srow = kw @ SB9 gives the per-pixel sum of present tap weights.
    """
    px = np.arange(HW)
    h, w = px // W, px % W
    out = np.zeros((9, HW), np.float32)
    for q in range(9):
        dh, dw = q // 3 - 1, q % 3 - 1
        ok = (h + dh >= 0) & (h + dh < H) & (w + dw >= 0) & (w + dw < W)
        out[q] = ok.astype(np.float32)
    return out


# ---------------------------------------------------------------- the program

def build_program(sw, sim_gelu_identity=False, n_cores=NCORES, debug=False):
    gelu_f = AF.Identity if sim_gelu_identity else AF.Gelu

    nc = bacc.Bacc("TRN2", target_bir_lowering=False, debug=False,
                   num_devices=n_cores)

    I = {}
    for name, shape, dt in [
        ("x", [HW, D], F32),
        ("xt", [D, HW], BF16), ("w1t", [D, C], BF16),
        ("b1r", [1, C], BF16),
        ("b1c", [128, CT], F32),
        ("g1c", [128, CT], F32), ("be1c", [128, CT], F32),
        ("g2c", [128, CT], F32), ("be2c", [128, CT], F32),
        ("aw1t", [C, 128], BF16), ("ab1c", [128, 1], F32),
        ("aw2t", [128, 9], BF16), ("ab2r", [1, 9], F32),
        ("caw1t", [C, 64], BF16), ("caw2t", [64, C], BF16),
        ("pwt", [C, D], BF16),
        ("g3r", [1, D], F32), ("be3r", [1, D], F32),
        ("sbr", [1, 1], F32),
    ]:
        I[name] = nc.dram_tensor(name, shape, dt, kind="ExternalInput")
    out_d = nc.dram_tensor("out", [HW, D], F32, kind="ExternalOutput")

    ident = nc.inline_tensor(np.eye(128, dtype=np.float32), name="cident")
    ident1 = nc.inline_tensor(np.ones((1, 1), np.float32), name="cident1")
    ones_row = nc.inline_tensor(np.ones((1, 128), np.float32), name="conesr")
    ones_row_bf = nc.inline_tensor(np.ones((1, 128), NP_BF16), name="conesrb")
    ones_col = nc.inline_tensor(np.ones((128, 1), np.float32), name="conesc")
    ones_col_bf = nc.inline_tensor(np.ones((128, 1), NP_BF16), name="conescb")
    masks_c = nc.inline_tensor(_stencil_masks().astype(NP_BF16), name="cmasks")
    spb_c = nc.inline_tensor(_spatial_bands(sw), name="cspb")
    sb9_c = nc.inline_tensor(_tap_counts().astype(NP_BF16), name="csb9")
    magic_c = nc.inline_tensor(
        np.full((128, 8), 0x5f3759df, np.uint32), name="cmagic")
    one_u32_c = nc.inline_tensor(np.full((128, 8), 1, np.uint32), name="cone32")

    with tile.TileContext(nc) as tc:
        _body(nc, tc, I, out_d, ident, ident1, ones_row, ones_row_bf,
              ones_col, ones_col_bf, masks_c, spb_c, sb9_c, magic_c,
              one_u32_c, gelu_f, n_cores)
    nc.compile()
    return nc


def _rsqrt_cols(nc, sb, magic, one32, v, n, tag):
    """DVE-only rsqrt of v (128, n) f32 (in place OK) -> returns (128, n).

    Bit-trick initial guess + 2 Newton steps; rel err ~5e-6.
    """
    y = sb.tile([128, n], F32, tag=tag + "y")
    t = sb.tile([128, n], F32, tag=tag + "t")
    yu = y[:].bitcast(U32)
    nc.vector.tensor_tensor(yu, v[:].bitcast(U32), one32[:, 0:n],
                            OP.logical_shift_right)
    nc.vector.tensor_tensor(yu, magic[:, 0:n], yu, OP.subtract)
    for _ in range(RSQRT_ITERS):
        nc.vector.tensor_tensor(t[:], y[:], y[:], OP.mult)
        nc.vector.tensor_tensor(t[:], t[:], v[:], OP.mult)
        nc.vector.tensor_scalar(t[:], t[:], -0.5, 1.5, OP.mult, OP.add)
        nc.vector.tensor_tensor(y[:], y[:], t[:], OP.mult)
    return y


def _body(nc, tc, I, out_d, ident, ident1, ones_row, ones_row_bf, ones_col,
          ones_col_bf, masks_c, spb_c, sb9_c, magic_c, one_u32_c, gelu_f,
          n_cores=NCORES):
    nb = n_cores * HW

    with tc.tile_pool(name="sb", bufs=1) as sb, \
         tc.tile_pool(name="sb2", bufs=1) as sb2, \
         tc.tile_pool(name="psb", bufs=3, space="PSUM") as psB, \
         tc.tile_pool(name="psh", bufs=2, space="PSUM") as psH, \
         tc.tile_pool(name="dram", bufs=6, space="DRAM") as dram:

        def load(name, shape, ap_in, dt=F32, pool=sb, view=None):
            t = pool.tile(shape, dt, tag=name)
            dst = t[:] if view is None else t[:].rearrange(*view[0], **view[1])
            nc.sync.dma_start(dst, ap_in)
            return t

        # ---------------- persistent SBUF tensors + loads
        xt_sb = sb.tile([128, 2 * HW], BF16, tag="xt_sb")
        w1t_sb = sb.tile([128, 2 * C], BF16, tag="w1t_sb")
        for k in range(2):
            nc.sync.dma_start(
                xt_sb[:, k * HW:(k + 1) * HW],
                I["xt"].ap().rearrange("(k p) n -> p k n", p=128)[:, k, :])
            nc.sync.dma_start(
                w1t_sb[:, k * C:(k + 1) * C],
                I["w1t"].ap().rearrange("(k p) n -> p k n", p=128)[:, k, :])
        b1rb = load("b1rb", [1, C], I["b1r"].ap(), dt=BF16)
        b1c = load("b1c", [128, CT], I["b1c"].ap())
        g1c = load("g1c", [128, CT], I["g1c"].ap())
        be1c = load("be1c", [128, CT], I["be1c"].ap())
        g2c = load("g2c", [128, CT], I["g2c"].ap())
        be2c = load("be2c", [128, CT], I["be2c"].ap())
        aw1t_sb = load("aw1t_sb", [128, 8 * 128],
                       I["aw1t"].ap().rearrange("(k p) n -> p k n", p=128),
                       dt=BF16, view=(("p (k n) -> p k n",), dict(k=8)))
        ab1c = load("ab1c", [128, 1], I["ab1c"].ap())
        aw2t_sb = load("aw2t_sb", [128, 9], I["aw2t"].ap(), dt=BF16)
        ab2r = load("ab2r", [1, 9], I["ab2r"].ap())
        caw1t_sb = load("caw1t_sb", [128, 8 * 64],
                        I["caw1t"].ap().rearrange("(k p) n -> p k n", p=128),
                        dt=BF16, view=(("p (k n) -> p k n",), dict(k=8)))
        caw2t_sb = load("caw2t_sb", [64, C], I["caw2t"].ap(), dt=BF16)
        paug = sb.tile([128, 8 * 260], BF16, tag="paug")
        nc.sync.dma_start(
            paug[:].rearrange("p (k n) -> p k n", n=260)[:, :, 0:D],
            I["pwt"].ap().rearrange("(k p) n -> p k n", p=128))
        nc.vector.memset(
            paug[:].rearrange("p (k n) -> p k n", n=260)[:, :, D:D + 1],
            1.0 / C)
        nc.vector.memset(
            paug[:].rearrange("p (k n) -> p k n", n=260)[:, :, D + 1:D + 2],
            0.0)
        g3r = load("g3r", [1, D], I["g3r"].ap())
        be3r = load("be3r", [1, D], I["be3r"].ap())
        sbr = load("sbr", [1, 1], I["sbr"].ap())
        tid = load("tid", [128, 128], ident.ap())
        tid1 = load("tid1", [1, 1], ident1.ap())
        tonesr = load("tonesr", [1, 128], ones_row.ap())
        tonesrb = load("tonesrb", [1, 128], ones_row_bf.ap(), dt=BF16)
        tonesc = load("tonesc", [128, 1], ones_col.ap())
        tonescb = load("tonescb", [128, 1], ones_col_bf.ap(), dt=BF16)
        xres = load("xres", [128, 8 * D],
                    I["x"].ap().rearrange("(t p) d -> p t d", p=128),
                    view=(("p (t d) -> p t d",), dict(t=8)))
        masks = load("masks", [128, 15 * 128], masks_c.ap(), dt=BF16)
        spb = load("spb", [128, 6 * 128], spb_c.ap())
        sb9 = load("sb9", [9, HW], sb9_c.ap(), dt=BF16)
        magic = load("magic", [128, 8], magic_c.ap(), dt=U32)
        one32 = load("one32", [128, 8], one_u32_c.ap(), dt=U32)

        # big working tensors (bf16)
        yg_sp = sb2.tile([128, HT * C], BF16, tag="ygsp")   # gelu1, spatial-major
        g2o = sb2.tile([128, CT * HW], BF16, tag="g2o")     # gelu2, ch-major
        yca = sb2.tile([128, CT * HW], BF16, tag="yca")     # ch-att out
        ygscr = [sb.tile([128, HW], BF16, tag=f"ygscr{i}", name=f"ygscr{i}")
                 for i in range(2)]
        sqscr = [sb.tile([128, HW], BF16, tag=f"sqscr{i}", name=f"sqscr{i}")
                 for i in range(2)]
        stat1l = sb.tile([128, 16], F32, tag="stat1l")
        stat1g = sb.tile([128, 16], F32, tag="stat1g")
        stat2l = sb.tile([128, 16], F32, tag="stat2l")
        stat2g = sb.tile([128, 16], F32, tag="stat2g")
        dscr = sb.tile([1, 1], F32, tag="dscr")             # ACT table preload dst

        # table preload: gelu load hides under the input DMAs
        nc.scalar.activation(dscr[:], tid1[:], gelu_f, bias=0.0, scale=1.0)

        # ============================ PHASE 1a: expand (ch-major) + stats1
        for m in range(CT):
            ps = psB.tile([128, HW], F32, tag="psb")
            for h in range(2):
                for k in range(2):
                    nc.tensor.matmul(
                        ps[:, h * 512:(h + 1) * 512],
                        w1t_sb[:, k * C + m * 128: k * C + (m + 1) * 128],
                        xt_sb[:, k * HW + h * 512: k * HW + (h + 1) * 512],
                        start=(k == 0), stop=(k == 1))
            yscr = ygscr[m % 2]
            nc.scalar.activation(
                yscr[:], ps[:], gelu_f, bias=b1c[:, m:m + 1], scale=1.0,
                accum_out=stat1l[:, m:m + 1])
            nc.vector.scalar_tensor_tensor(
                sqscr[m % 2][:], yscr[:], 0.0, yscr[:], OP.bypass, OP.mult,
                accum_out=stat1l[:, 8 + m:9 + m])

        # ============================ AG1 (BN1 batch stats)
        bb1i = dram.tile([128, 16], F32, tag="bb1i")
        bb1o = dram.tile([n_cores * 128, 16], F32, tag="bb1o")
        nc.gpsimd.dma_start(bb1i[:], stat1l[:])
        nc.gpsimd.collective_compute(
            "AllGather", OP.bypass, replica_groups=[list(range(n_cores))],
            ins=[bb1i.opt()], outs=[bb1o.opt()])

        # ============================ PHASE 1b: expand again, spatial-major
        # (runs on PE/ACT while the AG1 collective is in flight)
        for t in range(HT):
            ps2 = psB.tile([128, HW], F32, tag="psb")
            for g in range(2):
                for k in range(2):
                    nc.tensor.matmul(
                        ps2[:, g * 512:(g + 1) * 512],
                        xt_sb[:, k * HW + t * 128: k * HW + (t + 1) * 128],
                        w1t_sb[:, k * C + g * 512: k * C + (g + 1) * 512],
                        start=(k == 0), stop=False)
                nc.tensor.matmul(
                    ps2[:, g * 512:(g + 1) * 512],
                    tonesrb[:],
                    b1rb[:, g * 512:(g + 1) * 512],
                    start=False, stop=True)
            nc.scalar.activation(
                yg_sp[:, t * C:(t + 1) * C], ps2[:], gelu_f,
                bias=0.0, scale=1.0)
        # preload the Exp table while AG1 is still in flight
        nc.scalar.activation(dscr[:], tid1[:], AF.Exp, bias=0.0, scale=1.0)

        # gather AG1 result + local combine
        gath1 = sb.tile([128, n_cores * 16], F32, tag="gath1")
        nc.gpsimd.dma_start(
            gath1[:].rearrange("p (r f) -> p r f", f=16),
            bb1o[:].rearrange("(r p) f -> p r f", p=128))
        nc.vector.tensor_copy(stat1g[:], gath1[:, 0:16])
        for r in range(1, n_cores):
            nc.vector.tensor_tensor(stat1g[:], stat1g[:],
                                    gath1[:, r * 16:(r + 1) * 16], OP.add)

        # ============================ PHASE 3: BN1 affine + kw + L build
        def bn_affine(statg, gcol, becol, tagp):
            """-> (a, bn) per-channel scale/shift columns (128, CT)."""
            mns = sb.tile([128, CT], F32, tag=tagp + "m")
            var = sb.tile([128, CT], F32, tag=tagp + "v")
            a = sb.tile([128, CT], F32, tag=tagp + "a")
            bn = sb.tile([128, CT], F32, tag=tagp + "b")
            nc.vector.tensor_scalar_mul(mns[:], statg[:, 0:8], 1.0 / nb)
            nc.vector.tensor_tensor(var[:], mns[:], mns[:], OP.mult)
            nc.vector.scalar_tensor_tensor(
                var[:], statg[:, 8:16], 1.0 / nb, var[:], OP.mult, OP.subtract)
            nc.vector.tensor_scalar_add(var[:], var[:], EPS)
            rs = _rsqrt_cols(nc, sb, magic, one32, var, CT, tagp + "r")
            nc.vector.tensor_tensor(a[:], gcol[:], rs[:], OP.mult)
            nc.vector.tensor_tensor(bn[:], mns[:], a[:], OP.mult)
            nc.vector.tensor_tensor(bn[:], becol[:], bn[:], OP.subtract)
            return a, bn

        a1, b1n = bn_affine(stat1g, g1c, be1c, "s1")
        inva1 = sb.tile([128, CT], F32, tag="inva1")
        bpre = sb.tile([128, CT], F32, tag="bpre")
        nc.vector.reciprocal(inva1[:], a1[:])
        nc.vector.tensor_tensor(bpre[:], b1n[:], inva1[:], OP.mult)
        # b' row (1, C) via 8 tiny column transposes (no DRAM bounce)
        b1rowb = sb.tile([1, C], BF16, tag="b1rowb")
        for half in range(2):
            psb1 = psH.tile([1, 512], F32, tag="psh")
            for j in range(4):
                c = half * 4 + j
                nc.tensor.transpose(psb1[:, j * 128:(j + 1) * 128],
                                    bpre[:, c:c + 1], tid[:])
            nc.scalar.copy(b1rowb[:, half * 512:(half + 1) * 512], psb1[:])

        # gap (local, normalized) -> kw
        gapn = sb.tile([128, CT], F32, tag="gapn")
        gapb = sb.tile([128, CT], BF16, tag="gapb")
        nc.vector.scalar_tensor_tensor(
            gapn[:], stat1l[:, 0:8], 1.0 / HW, a1[:], OP.mult, OP.mult)
        nc.vector.tensor_tensor(gapn[:], gapn[:], b1n[:], OP.add)
        nc.vector.tensor_copy(gapb[:], gapn[:])

        ph1 = psH.tile([128, 1], F32, tag="psh")
        for k in range(CT):
            nc.tensor.matmul(ph1[:], aw1t_sb[:, k * 128:(k + 1) * 128],
                             gapb[:, k:k + 1], start=(k == 0), stop=(k == 7))
        h1 = sb.tile([128, 1], BF16, tag="h1")
        nc.vector.tensor_scalar(h1[:], ph1[:], ab1c[:], 0.0, OP.add, OP.max)
        ps9 = psH.tile([1, 9], F32, tag="psh")
        nc.tensor.matmul(ps9[:], h1[:], aw2t_sb[:], start=True, stop=True)
        v9 = sb.tile([1, 9], F32, tag="v9")
        nc.vector.tensor_tensor(v9[:], ps9[:], ab2r[:], OP.add)
        mx9 = sb.tile([1, 1], F32, tag="mx9")
        nc.vector.tensor_reduce(mx9[:], v9[:], mybir.AxisListType.X, OP.max)
        nc.vector.tensor_scalar(v9[:], v9[:], mx9[:], None, OP.subtract)
        e9 = sb.tile([1, 9], F32, tag="e9")
        se = sb.tile([1, 1], F32, tag="se")
        nc.scalar.activation(e9[:], v9[:], AF.Exp, bias=0.0, scale=1.0,
                             accum_out=se[:])
        # re-preload Gelu for phase 4 (hides under the kw/L-build chain)
        nc.scalar.activation(dscr[:], tid1[:], gelu_f, bias=0.0, scale=1.0)
        rse = sb.tile([1, 1], F32, tag="rse")
        nc.vector.reciprocal(rse[:], se[:])
        kw9 = sb.tile([1, 9], F32, tag="kw9")
        nc.vector.tensor_scalar(kw9[:], e9[:], rse[:], None, OP.mult)
        # broadcast kw to all partitions (for the L build scalars)
        pskb = psH.tile([128, 9], F32, tag="psh")
        nc.tensor.matmul(pskb[:], tonesr[:], kw9[:], start=True, stop=True)
        kwb = sb.tile([128, 9], F32, tag="kwb")
        nc.vector.tensor_copy(kwb[:], pskb[:])
        # kw as a column (9, 1) for the srow matmul
        pskc = psH.tile([9, 1], F32, tag="psh")
        nc.tensor.transpose(pskc[:], kw9[:], tid1[:])
        kwcol = sb.tile([9, 1], BF16, tag="kwcol")
        nc.scalar.copy(kwcol[:], pskc[:])

        # L band tiles, concatenated [L(-1) | L(0) | L(+1)]
        L = sb.tile([128, 3 * 128], BF16, tag="L")
        Ldm, Ld0, Ldp = L[:, 0:128], L[:, 128:256], L[:, 256:384]
        nc.vector.tensor_scalar(Ld0, masks[:, 3 * 128:4 * 128], kwb[:, 0:1],
                                None, OP.mult)
        for q in range(1, 9):
            nc.vector.scalar_tensor_tensor(
                Ld0, masks[:, (3 + q) * 128:(4 + q) * 128], kwb[:, q:q + 1],
                Ld0, OP.mult, OP.add)
        nc.vector.tensor_scalar(Ldm, masks[:, 0:128], kwb[:, 6:7], None, OP.mult)
        for i, q in enumerate((7, 8)):
            nc.vector.scalar_tensor_tensor(
                Ldm, masks[:, (1 + i) * 128:(2 + i) * 128], kwb[:, q:q + 1],
                Ldm, OP.mult, OP.add)
        nc.vector.tensor_scalar(Ldp, masks[:, 12 * 128:13 * 128], kwb[:, 0:1],
                                None, OP.mult)
        for i, q in enumerate((1, 2)):
            nc.vector.scalar_tensor_tensor(
                Ldp, masks[:, (13 + i) * 128:(14 + i) * 128], kwb[:, q:q + 1],
                Ldp, OP.mult, OP.add)

        # srow = kw @ SB9 (per-pixel sum of present taps)
        srowb = sb.tile([1, HW], BF16, tag="srowb")
        for h in range(2):
            pss = psH.tile([1, 512], F32, tag="psh")
            nc.tensor.matmul(pss[:], kwcol[:],
                             sb9[:, h * 512:(h + 1) * 512],
                             start=True, stop=True)
            nc.scalar.copy(srowb[:, h * 512:(h + 1) * 512], pss[:])

        # ============================ PHASE 4: stencil + gelu2 + stats2
        mxc = sb.tile([128, CT], F32, tag="mxc")
        for c in range(CT):
            psz = psB.tile([128, HW], F32, tag="psb")
            for h in range(2):
                nc.tensor.matmul(psz[:, h * 512:(h + 1) * 512],
                                 b1rowb[:, c * 128:(c + 1) * 128],
                                 srowb[:, h * 512:(h + 1) * 512],
                                 start=True, stop=False)
            for t_in in range(HT):
                lo = max(0, (t_in - 1) * 128)
                hi = min(HW, (t_in + 2) * 128)
                roff = 128 + (lo - t_in * 128)
                if lo < 512 < hi:
                    pieces = [(lo, 512), (512, hi)]
                else:
                    pieces = [(lo, hi)]
                for (a, b) in pieces:
                    ra = roff + (a - lo)
                    last_bank0 = (a < 512) and (t_in == 4)
                    last_bank1 = (a >= 512) and (t_in == 7)
                    nc.tensor.matmul(
                        psz[:, a:b],
                        yg_sp[:, t_in * C + c * 128: t_in * C + (c + 1) * 128],
                        L[:, ra:ra + (b - a)],
                        start=False, stop=(last_bank0 or last_bank1))
            nc.scalar.activation(
                g2o[:, c * HW:(c + 1) * HW], psz[:], gelu_f,
                bias=0.0, scale=a1[:, c:c + 1],
                accum_out=stat2l[:, c:c + 1])
            srcg2 = g2o[:, c * HW:(c + 1) * HW]
            nc.vector.scalar_tensor_tensor(
                sqscr[c % 2][:], srcg2, 0.0, srcg2, OP.bypass, OP.mult,
                accum_out=stat2l[:, 8 + c:9 + c])

        # ============================ AG2 (BN2 batch stats)
        bb2i = dram.tile([128, 16], F32, tag="bb2i")
        bb2o = dram.tile([n_cores * 128, 16], F32, tag="bb2o")
        nc.gpsimd.dma_start(bb2i[:], stat2l[:])
        nc.gpsimd.collective_compute(
            "AllGather", OP.bypass, replica_groups=[list(range(n_cores))],
            ins=[bb2i.opt()], outs=[bb2o.opt()])
        # per-channel max over HW on DVE, hidden under the AG2 flight
        for c in range(CT):
            nc.vector.tensor_reduce(mxc[:, c:c + 1],
                                    g2o[:, c * HW:(c + 1) * HW],
                                    mybir.AxisListType.X, OP.max)
        # preload Sigmoid while AG2 is in flight
        nc.scalar.activation(dscr[:], tid1[:], AF.Sigmoid, bias=0.0, scale=1.0)
        gath2 = sb.tile([128, n_cores * 16], F32, tag="gath2")
        nc.gpsimd.dma_start(
            gath2[:].rearrange("p (r f) -> p r f", f=16),
            bb2o[:].rearrange("(r p) f -> p r f", p=128))
        nc.vector.tensor_copy(stat2g[:], gath2[:, 0:16])
        for r in range(1, n_cores):
            nc.vector.tensor_tensor(stat2g[:], stat2g[:],
                                    gath2[:, r * 16:(r + 1) * 16], OP.add)

        # ============================ PHASE 6: BN2 + channel attention
        a2, b2n = bn_affine(stat2g, g2c, be2c, "s2")
        amx = sb.tile([128, 2 * CT], F32, tag="amx")
        amxb = sb.tile([128, 2 * CT], BF16, tag="amxb")
        nc.vector.scalar_tensor_tensor(
            amx[:, 0:8], stat2l[:, 0:8], 1.0 / HW, a2[:], OP.mult, OP.mult)
        nc.vector.tensor_tensor(amx[:, 0:8], amx[:, 0:8], b2n[:], OP.add)
        nc.vector.tensor_tensor(amx[:, 8:16], mxc[:], a2[:], OP.mult)
        nc.vector.tensor_tensor(amx[:, 8:16], amx[:, 8:16], b2n[:], OP.add)
        nc.vector.tensor_copy(amxb[:], amx[:])

        psf = psH.tile([64, 2], F32, tag="psh")
        for k in range(CT):
            nc.tensor.matmul(psf[:], caw1t_sb[:, k * 64:(k + 1) * 64],
                             amxb[:, k:k + 9:8], start=(k == 0), stop=(k == 7))
        hsum = sb.tile([64, 1], BF16, tag="hsum")
        hp = sb.tile([64, 2], F32, tag="hp")
        nc.vector.tensor_scalar(hp[:], psf[:], 0.0, None, OP.max)
        nc.vector.tensor_tensor(hsum[:], hp[:, 0:1], hp[:, 1:2], OP.add)

        psc = psH.tile([128, CT], F32, tag="psh")
        for c in range(CT):
            nc.tensor.matmul(psc[:, c:c + 1], caw2t_sb[:, c * 128:(c + 1) * 128],
                             hsum[:], start=True, stop=True)
        scol = sb.tile([128, CT], F32, tag="scol")
        nc.scalar.activation(scol[:], psc[:], AF.Sigmoid, bias=0.0, scale=1.0)

        sprime = sb.tile([128, CT], F32, tag="sprime")
        b2s = sb.tile([128, CT], F32, tag="b2s")
        b2sb = sb.tile([128, CT], BF16, tag="b2sb")
        nc.vector.tensor_tensor(sprime[:], scol[:], a2[:], OP.mult)
        nc.vector.tensor_tensor(b2s[:], scol[:], b2n[:], OP.mult)
        nc.vector.tensor_copy(b2sb[:], b2s[:])

        # y_ca (materialized for the channel-max)
        for c in range(CT):
            nc.vector.tensor_scalar(yca[:, c * HW:(c + 1) * HW],
                                    g2o[:, c * HW:(c + 1) * HW],
                                    sprime[:, c:c + 1], b2s[:, c:c + 1],
                                    OP.mult, OP.add)

        # scaled projection weights
        pws = sb2.tile([128, 8 * 260], BF16, tag="pws")
        for c in range(CT):
            nc.vector.tensor_scalar(pws[:, c * 260:c * 260 + 258],
                                    paug[:, c * 260:c * 260 + 258],
                                    sprime[:, c:c + 1], None, OP.mult)
        # t2 row (rank-1 bias of the projection)
        pst2 = psH.tile([1, 258], F32, tag="psh")
        for c in range(CT):
            nc.tensor.matmul(pst2[:], b2sb[:, c:c + 1],
                             paug[:, c * 260:c * 260 + 258],
                             start=(c == 0), stop=(c == 7))
        u2row = sb.tile([1, 258], BF16, tag="u2row")
        nc.scalar.copy(u2row[:], pst2[:])

        # projection -> proj_sb (spatial-major (hw, d))
        proj_sb = sb2.tile([128, 8 * 258], BF16, tag="proj_sb")
        avgpx = sb.tile([128, HT], F32, tag="avgpx")
        for mt in range(HT):
            psp = psH.tile([128, 258], F32, tag="psh")
            for c in range(CT):
                nc.tensor.matmul(psp[:],
                                 g2o[:, c * HW + mt * 128: c * HW + (mt + 1) * 128],
                                 pws[:, c * 260:c * 260 + 258],
                                 start=(c == 0), stop=False)
            nc.tensor.matmul(psp[:], tonesrb[:], u2row[:], start=False,
                             stop=True)
            dst = proj_sb[:, mt * 258:mt * 258 + 258]
            if mt % 2 == 0:
                nc.vector.tensor_copy(dst, psp[:])
            else:
                nc.scalar.copy(dst, psp[:])
            nc.vector.tensor_copy(avgpx[:, mt:mt + 1], psp[:, D:D + 1])

        # channel max (per pixel): in-place pairwise tree over yca, then a
        # partition all-reduce on GPSIMD
        for i in range(4):
            nc.vector.tensor_tensor(yca[:, (2 * i) * HW:(2 * i + 1) * HW],
                                    yca[:, (2 * i) * HW:(2 * i + 1) * HW],
                                    yca[:, (2 * i + 1) * HW:(2 * i + 2) * HW],
                                    OP.max)
        nc.vector.tensor_tensor(yca[:, 0:HW], yca[:, 0:HW],
                                yca[:, 2 * HW:3 * HW], OP.max)
        nc.vector.tensor_tensor(yca[:, 4 * HW:5 * HW], yca[:, 4 * HW:5 * HW],
                                yca[:, 6 * HW:7 * HW], OP.max)
        nc.vector.tensor_tensor(yca[:, 0:HW], yca[:, 0:HW],
                                yca[:, 4 * HW:5 * HW], OP.max)
        mxbc = sb2.tile([128, HW], F32, tag="mxbc")
        nc.gpsimd.partition_all_reduce(mxbc[:], yca[:, 0:HW], 128,
                                       bass_isa.ReduceOp.max)
        # row 0 of mxbc = per-pixel channel max; to columns via PE transposes
        tid1b = tid1
        mxpx = sb.tile([128, HT], F32, tag="mxpx")
        psmx = psH.tile([128, HT], F32, tag="psh")
        for t in range(HT):
            nc.tensor.transpose(psmx[:, t:t + 1],
                                mxbc[0:1, t * 128:(t + 1) * 128], tid1b[:])
        nc.vector.tensor_copy(mxpx[:], psmx[:])

        # sb broadcast column
        pssb = psH.tile([128, 1], F32, tag="psh")
        nc.tensor.matmul(pssb[:], tonesr[:], sbr[:], start=True, stop=True)
        sbc = sb.tile([128, 1], F32, tag="sbc")
        nc.scalar.copy(sbc[:], pssb[:])

        # spatial 7x7 conv as 6 shifted-column matmuls (2 ch x 3 bands)
        pssp = psH.tile([128, HT], F32, tag="psh")
        mmspecs = []
        for ch, srccol in ((0, avgpx), (1, mxpx)):
            mmspecs.append((ch * 3 + 1, slice(0, 8), srccol[:, 0:8]))
            mmspecs.append((ch * 3 + 2, slice(1, 8), srccol[:, 0:7]))
            mmspecs.append((ch * 3 + 0, slice(0, 7), srccol[:, 1:8]))
        for i, (bi, osl, rhs) in enumerate(mmspecs):
            nc.tensor.matmul(pssp[:, osl], spb[:, bi * 128:(bi + 1) * 128],
                             rhs, start=(i == 0), stop=(i == len(mmspecs) - 1))
        spcol = sb.tile([128, HT], F32, tag="spcol")
        spcolb = sb.tile([128, HT], BF16, tag="spcolb")
        nc.scalar.activation(spcol[:], pssp[:], AF.Sigmoid, bias=sbc[:],
                             scale=1.0)
        nc.vector.tensor_copy(spcolb[:], spcol[:])

        # spp = proj * sp (spatial scale, per-partition)
        spp = sb2.tile([128, 8 * 258], BF16, tag="spp")
        for mt in range(HT):
            nc.vector.tensor_scalar(spp[:, mt * 258:mt * 258 + 256],
                                    proj_sb[:, mt * 258:mt * 258 + 256],
                                    spcol[:, mt:mt + 1], None, OP.mult)

        # BN3 stats: sum(sp*proj) and sum((sp*proj)^2) over hw
        pst3a = psH.tile([1, D], F32, tag="psh")
        for mt in range(HT):
            nc.tensor.matmul(pst3a[:], spcolb[:, mt:mt + 1],
                             proj_sb[:, mt * 258:mt * 258 + 256],
                             start=(mt == 0), stop=(mt == 7))
        pst3b = psH.tile([1, D], F32, tag="psh")
        sqs = sb.tile([128, 2 * D], BF16, tag="sqs")
        for mt in range(HT):
            half = (mt % 2) * D
            src = spp[:, mt * 258:mt * 258 + 256]
            nc.vector.scalar_tensor_tensor(
                sqs[:, half:half + D], src, 0.0, src, OP.bypass, OP.mult)
            nc.tensor.matmul(pst3b[:], tonescb[:], sqs[:, half:half + D],
                             start=(mt == 0), stop=(mt == 7))
        stat3l = sb.tile([1, 2 * D], F32, tag="stat3l")
        nc.scalar.copy(stat3l[:, 0:D], pst3a[:])
        nc.vector.tensor_copy(stat3l[:, D:2 * D], pst3b[:])

        # ============================ AG3 (BN3 batch stats)
        bb3i = dram.tile([1, 2 * D], F32, tag="bb3i")
        bb3o = dram.tile([n_cores, 2 * D], F32, tag="bb3o")
        nc.gpsimd.dma_start(bb3i[:], stat3l[:])
        nc.gpsimd.collective_compute(
            "AllGather", OP.bypass, replica_groups=[list(range(n_cores))],
            ins=[bb3i.opt()], outs=[bb3o.opt()])
        # preload Sqrt for the BN3 affine while AG3 is in flight
        nc.scalar.activation(dscr[:], tid1[:], AF.Sqrt, bias=0.0, scale=1.0)
        gath3 = sb.tile([n_cores, 2 * D], F32, tag="gath3")
        nc.gpsimd.dma_start(gath3[:], bb3o[:])
        pst3g = psH.tile([1, 2 * D], F32, tag="psh")
        nc.tensor.matmul(pst3g[:], tonesc[0:n_cores, :], gath3[:],
                         start=True, stop=True)
        stat3g = sb.tile([1, 2 * D], F32, tag="stat3g")
        nc.scalar.copy(stat3g[:], pst3g[:])

        # BN3 affine in row form (pb cancels through the mean subtraction)
        m3 = sb.tile([1, D], F32, tag="m3")
        v3 = sb.tile([1, D], F32, tag="v3")
        a3r = sb.tile([1, D], F32, tag="a3r")
        c3r = sb.tile([1, D], F32, tag="c3r")
        tmp3 = sb.tile([1, D], F32, tag="tmp3")
        nc.vector.tensor_scalar_mul(m3[:], stat3g[:, 0:D], 1.0 / nb)
        nc.vector.tensor_tensor(tmp3[:], m3[:], m3[:], OP.mult)
        nc.vector.scalar_tensor_tensor(
            v3[:], stat3g[:, D:2 * D], 1.0 / nb, tmp3[:], OP.mult, OP.subtract)
        nc.vector.tensor_scalar_add(v3[:], v3[:], EPS)
        nc.scalar.sqrt(v3[:], v3[:])
        nc.vector.reciprocal(v3[:], v3[:])
        nc.vector.tensor_tensor(a3r[:], g3r[:], v3[:], OP.mult)
        nc.vector.tensor_tensor(tmp3[:], a3r[:], m3[:], OP.mult)
        nc.vector.tensor_tensor(c3r[:], be3r[:], tmp3[:], OP.subtract)

        # broadcast a3/c3 to all partitions
        a3b = sb.tile([128, D], F32, tag="a3b")
        c3b = sb.tile([128, D], F32, tag="c3b")
        for rowt, dstt in ((a3r, a3b), (c3r, c3b)):
            psx = psH.tile([128, D], F32, tag="psh")
            nc.tensor.matmul(psx[:], tonesr[:], rowt[:], start=True, stop=True)
            nc.vector.tensor_copy(dstt[:], psx[:])

        # final: out = (x + c3) + spp*a3
        # (gpsimd cannot help here: its ucode library must stay on `attn`
        # for partition_all_reduce — TensorTensor lives in a different,
        # mutually-exclusive library)
        out_sb = sb2.tile([128, 8 * D], F32, tag="outsb")
        scl = sb2.tile([128, 8 * D], F32, tag="scl")
        for mt in range(HT):
            sl = slice(mt * D, (mt + 1) * D)
            ssl = spp[:, mt * 258:mt * 258 + 256]
            nc.vector.tensor_tensor(out_sb[:, sl], xres[:, sl], c3b[:], OP.add)
            nc.vector.tensor_tensor(scl[:, sl], ssl, a3b[:], OP.mult)
            nc.vector.tensor_tensor(out_sb[:, sl], out_sb[:, sl], scl[:, sl],
                                    OP.add)
            nc.sync.dma_start(
                out_d.ap().rearrange("(t p) d -> p t d", p=128)[:, mt, :],
                out_sb[:, mt * D:(mt + 1) * D])


# ---------------------------------------------------------------- host driver

def shard_inputs(inputs):
    """Full inputs -> per-core in_maps (host-side layout staging only)."""
    x = np.ascontiguousarray(np.asarray(inputs["x"], np.float32))
    w1 = np.asarray(inputs["w1"], np.float32)
    f32 = lambda a: np.ascontiguousarray(a).astype(np.float32)
    bf = lambda a: np.ascontiguousarray(a).astype(NP_BF16)
    shared = {
        "w1t": bf(w1.T),
        "b1r": bf(inputs["b1"].reshape(1, C)),
        "b1c": f32(inputs["b1"].reshape(CT, 128).T),
        "g1c": f32(inputs["g1"].reshape(CT, 128).T),
        "be1c": f32(inputs["be1"].reshape(CT, 128).T),
        "g2c": f32(inputs["g2"].reshape(CT, 128).T),
        "be2c": f32(inputs["be2"].reshape(CT, 128).T),
        "aw1t": bf(np.asarray(inputs["aw1"], np.float32).T),
        "ab1c": f32(inputs["ab1"].reshape(1, 128).T),
        "aw2t": bf(np.asarray(inputs["aw2"], np.float32).T),
        "ab2r": f32(inputs["ab2"].reshape(1, 9)),
        "caw1t": bf(np.asarray(inputs["ca_w1"], np.float32).T),
        "caw2t": bf(np.asarray(inputs["ca_w2"], np.float32).T),
        "pwt": bf(np.asarray(inputs["pw"], np.float32).T),
        "g3r": f32(inputs["g3"].reshape(1, D)),
        "be3r": f32(inputs["be3"].reshape(1, D)),
        "sbr": f32(inputs["sb"].reshape(1, 1)),
    }
    in_maps = []
    for i in range(NCORES):
        m = dict(shared)
        m["x"] = np.ascontiguousarray(x[i])
        m["xt"] = bf(x[i].T)
        in_maps.append(m)
    return in_maps


_CACHE = {}


def get_program(sw, sim_gelu_identity=False, n_cores=NCORES, debug=False):
    key = ("sim" if sim_gelu_identity else "hw", n_cores, debug, sw.tobytes())
    if key not in _CACHE:
        _CACHE[key] = build_program(sw, sim_gelu_identity=sim_gelu_identity,
                                    n_cores=n_cores, debug=debug)
    return _CACHE[key]


def run(inputs, trace=False):
    nc = get_program(np.asarray(inputs["sw"], np.float32))
    in_maps = shard_inputs(inputs)
    r = bass_utils.run_bass_kernel_spmd(
        nc, in_maps, core_ids=list(range(NCORES)), trace=trace)
    out = np.stack([r.results[i]["out"] for i in range(NCORES)], axis=0)
    return out.astype(np.float32), r


def kernel(**inputs) -> np.ndarray:
    out, _ = run(inputs, trace=False)
    return out
